# revision 1
# baseline (speedup 1.0000x reference)
"""HGAT message-passing kernel for Trainium2 (8 NeuronCores, SPMD).

Reference computation (B=4, N=4096, C_IN=128, C_OUT=64):
    h   = node_rep @ proj_W.T + proj_b                    # [B,N,64]
    f1  = rowsum(h * k_W[node_type]) + k_b[node_type]     # [B,N]
    f2  = rowsum(h * v_W[node_type]) + v_b[node_type]     # [B,N]
    L   = adj[i,j] * (f1[i] + f2[j])
    u   = sigmoid(L) - 0.5
    P   = softmax(u, axis=i)      # normalized over rows i, per column j
    out = P @ h                   # contract over j

Key algebra used on device:
  * softmax-over-i / contract-over-j means out = E @ (h / colsum) with
    E[i,j] = exp(sigmoid(L)) and colsum[j] = sum_i E[i,j]; the -0.5 and the
    softmax max-subtraction cancel in the ratio.
  * sigmoid(x) = 0.5 + 0.5*tanh(0.5 x); tanh and exp share one ACT table set.
  * exp's accum_out produces colsum for free.

Sharding: core c handles batch b=c//2 and j-half h=c%2 (rows of adj.T).
Host pre-transposes adj (so the device contracts over j on the partition
axis), gathers k_W/v_W rows by node_type (pure data movement), and sums the
two per-batch partial outputs at the end.
"""

import os
import sys

import numpy as np

sys.path.insert(0, "/opt/trn_rl_repo")

import concourse.bass as bass  # noqa: E402
import concourse.tile as tile  # noqa: E402
from concourse import bacc  # noqa: E402
from concourse import mybir  # noqa: E402
from concourse.bass_utils import run_bass_kernel_spmd  # noqa: E402

B = 4
N = 4096
CIN = 128
COUT = 64
P = 128                      # SBUF partitions
NJ = N // 2                  # j rows per core (adjacency half)
NJT = NJ // P                # 16 j-tiles per core
NIC = N // 512               # 8 i-chunks of 512
NIT = N // P                 # 32 i-chunks of 128

F32 = mybir.dt.float32
AF = mybir.ActivationFunctionType
ALU = mybir.AluOpType

# dtype for the attention tensor + h operand of the final matmul.
# f32 is exact-ish; bfloat16 halves PE time if needed for perf.
ET_DTYPE = F32

LAST_EXEC_NS = None
LAST_RESULTS = None


def build_nc(n=N, nj=NJ, et_dtype=None):
    """Build the single-core SPMD Bass program (same program on all cores)."""
    if et_dtype is None:
        et_dtype = ET_DTYPE
    # fp32 path: produce matmul operands as float32r (TF32-like, 4x faster
    # than fp32 on the PE). The verifier requires producers to round to f32r.
    mm_dtype = mybir.dt.float32r if et_dtype == F32 else et_dtype
    njt = nj // P
    nic = n // 512
    nit = n // P

    nc = bacc.Bacc()
    adjt_d = nc.dram_tensor("adjt", [nj, n], F32, kind="ExternalInput")
    xt_d = nc.dram_tensor("xt", [CIN, n], F32, kind="ExternalInput")
    xth_d = nc.dram_tensor("xth", [CIN, nj], F32, kind="ExternalInput")
    wpt_d = nc.dram_tensor("wpt", [CIN, COUT], F32, kind="ExternalInput")
    bpcol_d = nc.dram_tensor("bpcol", [COUT, 1], F32, kind="ExternalInput")
    bpb_d = nc.dram_tensor("bpb", [P, COUT], F32, kind="ExternalInput")
    kwt_d = nc.dram_tensor("kwt", [COUT, n], F32, kind="ExternalInput")
    kbrow_d = nc.dram_tensor("kbrow", [1, n], F32, kind="ExternalInput")
    vwn_d = nc.dram_tensor("vwn", [P, njt * COUT], F32, kind="ExternalInput")
    vbcol_d = nc.dram_tensor("vbcol", [P, njt], F32, kind="ExternalInput")
    outp_d = nc.dram_tensor("outp", [P, nit * COUT], F32, kind="ExternalOutput")

    with tile.TileContext(nc) as tc:
        with (
            tc.tile_pool(name="adjp", bufs=3) as adjp,
            tc.tile_pool(name="workp", bufs=2) as workp,
            tc.tile_pool(name="etp", bufs=2) as etp,
            tc.tile_pool(name="singles", bufs=1) as singles,
            tc.tile_pool(name="smalls", bufs=3) as smalls,
            tc.tile_pool(name="stream", bufs=2) as stream,
            tc.tile_pool(name="dscratch", bufs=1, space="DRAM") as dscratch,
            tc.tile_pool(name="psA", bufs=2, space="PSUM") as psA,
            tc.tile_pool(name="psO", bufs=1, space="PSUM") as psO,
        ):
            # ---------------- small parameter loads ----------------
            wpt_s = singles.tile([CIN, COUT], F32)
            nc.sync.dma_start(wpt_s, wpt_d[:, :])
            bpcol_s = singles.tile([COUT, 1], F32)
            nc.sync.dma_start(bpcol_s, bpcol_d[:, :])
            bpb_s = singles.tile([P, COUT], F32)
            nc.sync.dma_start(bpb_s, bpb_d[:, :])
            vbcol_s = singles.tile([P, njt], F32)
            nc.sync.dma_start(vbcol_s, vbcol_d[:, :])

            ones64 = singles.tile([COUT, 1], F32)
            nc.vector.memset(ones64, 1.0)
            zero_col = singles.tile([P, 1], F32)
            nc.vector.memset(zero_col, 0.0)
            half_col = singles.tile([P, 1], F32)
            nc.vector.memset(half_col, 0.5)

            # ------- f1 row, streamed in 512-col chunks through small tiles -------
            # f1[i] = sum_o (x@Wp.T + bp)[i,o] * KW[i,o] + kb[i]
            f1s = dscratch.tile([1, n], F32)
            for ic in range(nic):
                sl = slice(ic * 512, (ic + 1) * 512)
                xtc = stream.tile([CIN, 512], F32, tag="xtc")
                nc.sync.dma_start(xtc, xt_d[:, sl])
                psh = psA.tile([COUT, 512], F32, tag="ps")
                nc.tensor.matmul(psh, lhsT=wpt_s, rhs=xtc, start=True, stop=True)
                hTc = stream.tile([COUT, 512], F32, tag="hTc")
                nc.vector.tensor_scalar_add(hTc, psh, bpcol_s)
                kwc = stream.tile([COUT, 512], F32, tag="kwc")
                nc.sync.dma_start(kwc, kwt_d[:, sl])
                nc.vector.tensor_mul(hTc, hTc, kwc)
                psf = psA.tile([1, 512], F32, tag="ps", padded_shape=[128, 512])
                nc.tensor.matmul(psf, lhsT=ones64, rhs=hTc, start=True, stop=True)
                kbc = stream.tile([1, 512], F32, tag="kbc")
                nc.sync.dma_start(kbc, kbrow_d[:, sl])
                f1rc = stream.tile([1, 512], F32, tag="f1rc")
                nc.vector.tensor_add(f1rc, psf, kbc)
                nc.sync.dma_start(f1s[:, sl], f1rc)
            # broadcast f1 across all 128 partitions via DRAM round-trip
            f1b = singles.tile([P, n], F32)
            f1s_bcast = bass.AP(tensor=f1s.tensor, offset=f1s.offset, ap=[[0, P], [1, n]])
            nc.sync.dma_start(f1b, f1s_bcast)

            # ------- h natural (j-half nodes) for f2 and g, streamed -------
            hn = singles.tile([P, njt * COUT], F32)
            f2c = singles.tile([P, njt], F32)
            for t in range(njt):
                osl = slice(t * COUT, (t + 1) * COUT)
                xthc = stream.tile([CIN, P], F32, tag="xthc")
                nc.sync.dma_start(xthc, xth_d[:, t * P:(t + 1) * P])
                psn = psA.tile([P, COUT], F32, tag="ps", padded_shape=[128, 512])
                nc.tensor.matmul(psn, lhsT=xthc, rhs=wpt_s, start=True, stop=True)
                nc.vector.tensor_add(hn[:, osl], psn, bpb_s)
                vwc = stream.tile([P, COUT], F32, tag="vwc")
                nc.sync.dma_start(vwc, vwn_d[:, osl])
                pvc = stream.tile([P, COUT], F32, tag="pvc")
                nc.vector.tensor_mul(pvc, hn[:, osl], vwc)
                nc.vector.tensor_reduce(
                    f2c[:, t:t + 1], pvc, axis=mybir.AxisListType.X, op=ALU.add
                )
            f2cb = singles.tile([P, njt], F32)
            nc.vector.tensor_add(f2cb, f2c, vbcol_s)

            # ---------------- main loop over j-tiles ----------------
            # Natural-layout accumulator out[i, c] packed as [128, nit*64]
            # (4 PSUM banks). start=True clears has_written for a whole bank,
            # so interleaved 64-col accumulation groups are illegal; instead a
            # K=1 dummy matmul zeroes each bank once (start=True, full-bank
            # write sets has_written everywhere) and every real matmul
            # accumulates with start=False.
            ps_out = psO.tile([P, nit * COUT], F32)
            zw = min(512, nit * COUT)
            zt = singles.tile([1, zw], F32)
            nc.vector.memset(zt, 0.0)
            for k in range(0, nit * COUT, zw):
                nc.tensor.matmul(
                    ps_out[:, k:k + zw], lhsT=zt[:, 0:P], rhs=zt,
                    start=True, stop=False,
                )

            for jt in range(njt):
                adjt_t = adjp.tile([P, n], F32, tag="adj")
                nc.sync.dma_start(adjt_t, adjt_d[jt * P:(jt + 1) * P, :])

                # L[j,i] = (f1[i] + f2[j]) * adjT[j,i] — one fused DVE pass,
                # written in place over the adjacency tile.
                nc.vector.scalar_tensor_tensor(
                    adjt_t, f1b, f2cb[:, jt:jt + 1], adjt_t, op0=ALU.add, op1=ALU.mult
                )
                # t = tanh(L/2);  E = exp(t/2 + 1/2) = exp(sigmoid(L))
                tt = workp.tile([P, n], F32, tag="tt")
                nc.scalar.activation(tt, adjt_t, AF.Tanh, bias=zero_col, scale=0.5)
                et = etp.tile([P, n], mm_dtype, tag="et")
                cs = smalls.tile([P, 1], F32, tag="cs")
                nc.scalar.activation(et, tt, AF.Exp, bias=half_col, scale=0.5, accum_out=cs)

                rc = smalls.tile([P, 1], F32, tag="rc")
                nc.vector.reciprocal(rc, cs)
                g = smalls.tile([P, COUT], mm_dtype, tag="g")
                nc.vector.tensor_scalar_mul(g, hn[:, jt * COUT:(jt + 1) * COUT], rc)

                for it in range(nit):
                    # last matmul touching this bank closes its group
                    last = (jt == njt - 1) and (
                        ((it + 1) * COUT) % 512 == 0 or it == nit - 1
                    )
                    nc.tensor.matmul(
                        ps_out[:, it * COUT:(it + 1) * COUT],
                        lhsT=et[:, it * P:(it + 1) * P],
                        rhs=g,
                        start=False,
                        stop=last,
                    )

            out_sb = singles.tile([P, nit * COUT], F32)
            nc.vector.tensor_copy(out_sb, ps_out)
            nc.sync.dma_start(outp_d[:, :], out_sb)

    nc.finalize()
    return nc


def _prep_in_maps(node_rep, adj_matrix, node_type, proj_W, proj_b, k_W, k_b, v_W, v_b):
    """Host-side shard prep (data movement / layout only, no FLOPs on the model math)."""
    f32 = np.float32
    node_rep = np.ascontiguousarray(np.asarray(node_rep, dtype=f32))
    adj = np.ascontiguousarray(np.asarray(adj_matrix, dtype=f32))
    nt = np.asarray(node_type).astype(np.int64) % 5
    proj_W = np.asarray(proj_W, dtype=f32)
    proj_b = np.asarray(proj_b, dtype=f32)
    k_W = np.asarray(k_W, dtype=f32)
    k_b = np.asarray(k_b, dtype=f32)
    v_W = np.asarray(v_W, dtype=f32)
    v_b = np.asarray(v_b, dtype=f32)

    adjT = np.ascontiguousarray(adj.T)                      # adjT[j, i] = adj[i, j]
    wpt = np.ascontiguousarray(proj_W.T)                    # [CIN, COUT]
    bpcol = np.ascontiguousarray(proj_b[:, None])           # [COUT, 1]
    bpb = np.ascontiguousarray(np.broadcast_to(proj_b[None, :], (P, COUT)))
    KW = k_W[nt]                                            # [N, COUT] gather
    kwt = np.ascontiguousarray(KW.T)                        # [COUT, N]
    kbrow = np.ascontiguousarray(k_b[nt][None, :])          # [1, N]
    VW = v_W[nt]                                            # [N, COUT]
    vb = v_b[nt]                                            # [N]

    in_maps = []
    for core in range(8):
        b, half = divmod(core, 2)
        jsl = slice(half * NJ, (half + 1) * NJ)
        xT = np.ascontiguousarray(node_rep[b].T)            # [CIN, N]
        vw_h = VW[jsl]                                      # [NJ, COUT]
        vwn = np.ascontiguousarray(
            vw_h.reshape(NJT, P, COUT).transpose(1, 0, 2).reshape(P, NJT * COUT)
        )
        vbcol = np.ascontiguousarray(vb[jsl].reshape(NJT, P).T)  # [P, NJT]
        in_maps.append({
            "adjt": np.ascontiguousarray(adjT[jsl, :]),
            "xt": xT,
            "xth": np.ascontiguousarray(xT[:, jsl]),
            "wpt": wpt,
            "bpcol": bpcol,
            "bpb": bpb,
            "kwt": kwt,
            "kbrow": kbrow,
            "vwn": vwn,
            "vbcol": vbcol,
        })
    return in_maps


def kernel(node_rep, adj_matrix, node_type, proj_W, proj_b, k_W, k_b, v_W, v_b):
    global LAST_EXEC_NS, LAST_RESULTS
    in_maps = _prep_in_maps(
        node_rep, adj_matrix, node_type, proj_W, proj_b, k_W, k_b, v_W, v_b
    )
    nc = build_nc()
    trace = os.environ.get("KERNEL_TRACE", "0") == "1"
    res = run_bass_kernel_spmd(nc, in_maps, core_ids=list(range(8)), trace=trace)
    LAST_EXEC_NS = res.exec_time_ns
    LAST_RESULTS = res

    out = np.empty((B, N, COUT), dtype=np.float32)
    for b in range(B):
        acc = None
        for half in range(2):
            part = np.asarray(res.results[2 * b + half]["outp"], dtype=np.float32)
            acc = part if acc is None else acc + part
        out[b] = acc.reshape(P, NIT, COUT).transpose(1, 0, 2).reshape(N, COUT)
    return out



# revision 3
# speedup vs baseline: 1.3261x; 1.3261x over previous
"""HGAT message-passing kernel for Trainium2 (8 NeuronCores, SPMD).

Reference computation (B=4, N=4096, C_IN=128, C_OUT=64):
    h   = node_rep @ proj_W.T + proj_b                    # [B,N,64]
    f1  = rowsum(h * k_W[node_type]) + k_b[node_type]     # [B,N]
    f2  = rowsum(h * v_W[node_type]) + v_b[node_type]     # [B,N]
    L   = adj[i,j] * (f1[i] + f2[j])
    u   = sigmoid(L) - 0.5
    P   = softmax(u, axis=i)      # normalized over rows i, per column j
    out = P @ h                   # contract over j

Device algebra:
  * softmax over i / contract over j => out = E @ (h / colsum(E)) with
    E = exp(sigmoid(L)); the -0.5 and max-subtraction cancel in the ratio.
  * E is approximated by ONE activation-table pass:
        E ~ ALPHA + BETA * tanh(GAMMA * L + DELTA)   (max rel err 5.5e-4)
    The affine part folds away: with t = tanh(GAMMA*L+DELTA),
        colsum_j = ALPHA*N + BETA * sum_i t[i,j]     (tanh accum_out)
        out[i,c] = sum_j t[j->i]*q[j,c] + (ALPHA/BETA)*sum_j q[j,c]
    where q[j,c] = h[j,c] / (ALPHA*N/BETA + sum_i t[i,j]).  The rank-1
    correction rides the PSUM->SBUF drain as a per-partition scalar add.
  * f1 via folded weights G = k_W @ proj_W (host, 5x64x128 param-only):
    f1[i] = sum_c x[c,i]*Gg[c,i] + kb'[i]; the all-ones matmul reduces
    over c AND broadcasts across the 128 partitions in one PE pass.

Sharding: core c owns j-rows [c*512, (c+1)*512) of the (transposed)
adjacency for ALL batches and all i. The adjacency slice is loaded once
(bf16) and reused across the 4 batches; no collectives. Each core emits
a full [64, B*N] partial of out^T; host sums the 8 partials.
"""

import os
import sys

import numpy as np

sys.path.insert(0, "/opt/trn_rl_repo")

import ml_dtypes  # noqa: E402

import concourse.bass as bass  # noqa: E402,F401
import concourse.tile as tile  # noqa: E402
from concourse import bacc  # noqa: E402
from concourse import mybir  # noqa: E402
from concourse.bass_utils import run_bass_kernel_spmd  # noqa: E402

B = 4
N = 4096
CIN = 128
COUT = 64
P = 128
NJ = N // 8            # 512 j-rows per core
NJT = NJ // P          # 4 j-tiles per core

# exp(sigmoid(x)) ~ ALPHA + BETA*tanh(GAMMA*x + DELTA), max rel err 5.5e-4
ALPHA = 1.8590168
BETA = 0.85887245
GAMMA = 0.5083613
DELTA = -0.2499283
C0 = ALPHA * N / BETA      # denom' = C0 + sum_i t   (q = h/denom')
AB = ALPHA / BETA          # rank-1 scale

F32 = mybir.dt.float32
BF16 = mybir.dt.bfloat16
AF = mybir.ActivationFunctionType
ALU = mybir.AluOpType
NPBF = ml_dtypes.bfloat16

LAST_EXEC_NS = None
LAST_RESULTS = None


def build_nc():
    nc = bacc.Bacc()
    adjt_d = nc.dram_tensor("adjt", [NJ, N], BF16, kind="ExternalInput")
    xtall_d = nc.dram_tensor("xtall", [CIN, B * N], BF16, kind="ExternalInput")
    xthall_d = nc.dram_tensor("xthall", [CIN, B * NJ], BF16, kind="ExternalInput")
    gg_d = nc.dram_tensor("gg", [CIN, N], BF16, kind="ExternalInput")
    kbrow_d = nc.dram_tensor("kbrow", [1, N], BF16, kind="ExternalInput")
    wpt_d = nc.dram_tensor("wpt", [CIN, COUT], BF16, kind="ExternalInput")
    bpb_d = nc.dram_tensor("bpb", [P, COUT], F32, kind="ExternalInput")
    vwn_d = nc.dram_tensor("vwn", [P, NJT * COUT], BF16, kind="ExternalInput")
    vbcol_d = nc.dram_tensor("vbcol", [P, NJT], F32, kind="ExternalInput")
    outp_d = nc.dram_tensor("outp", [COUT, B * N], BF16, kind="ExternalOutput")

    with tile.TileContext(nc) as tc:
        with (
            tc.tile_pool(name="singles", bufs=1) as singles,
            tc.tile_pool(name="xtp", bufs=2) as xtp,
            tc.tile_pool(name="workp", bufs=2) as workp,
            tc.tile_pool(name="f1bp", bufs=2) as f1bp,
            tc.tile_pool(name="lp", bufs=2) as lp,
            tc.tile_pool(name="tp", bufs=6) as tp,
            tc.tile_pool(name="hnp", bufs=2) as hnp,
            tc.tile_pool(name="qp", bufs=2) as qp,
            tc.tile_pool(name="f2p", bufs=2) as f2p,
            tc.tile_pool(name="osb", bufs=2) as osb,
            tc.tile_pool(name="smalls", bufs=4) as smalls,
            tc.tile_pool(name="psf1b", bufs=2, space="PSUM") as psf1b,
            tc.tile_pool(name="psn", bufs=1, space="PSUM") as psnp,
            tc.tile_pool(name="psg", bufs=1, space="PSUM") as psgp,
            tc.tile_pool(name="pso", bufs=2, space="PSUM") as psop,
        ):
            # ---------------- persistent loads ----------------
            adjt_t = []
            for k in range(NJT):
                a = singles.tile([P, N], BF16, tag=f"adj{k}")
                nc.sync.dma_start(a, adjt_d[k * P:(k + 1) * P, :])
                adjt_t.append(a)
            gg_s = singles.tile([CIN, N], BF16)
            nc.sync.dma_start(gg_s, gg_d[:, :])
            xthall_s = singles.tile([CIN, B * NJ], BF16)
            nc.sync.dma_start(xthall_s, xthall_d[:, :])
            wpt_s = singles.tile([CIN, COUT], BF16)
            nc.sync.dma_start(wpt_s, wpt_d[:, :])
            bpb_s = singles.tile([P, COUT], F32)
            nc.sync.dma_start(bpb_s, bpb_d[:, :])
            vwn_s = singles.tile([P, NJT * COUT], BF16)
            nc.sync.dma_start(vwn_s, vwn_d[:, :])
            vbcol_s = singles.tile([P, NJT], F32)
            nc.sync.dma_start(vbcol_s, vbcol_d[:, :])
            kb_s = singles.tile([1, N], BF16)
            nc.sync.dma_start(kb_s, kbrow_d[:, :])

            ones128 = singles.tile([P, P], BF16)
            nc.vector.memset(ones128, 1.0)
            ones1 = singles.tile([1, P], BF16)
            nc.vector.memset(ones1, 1.0)
            onescol = singles.tile([P, 1], BF16)
            nc.vector.memset(onescol, 1.0)
            delta_col = singles.tile([P, 1], F32)
            nc.vector.memset(delta_col, DELTA)

            for b in range(B):
                bsl = slice(b * N, (b + 1) * N)
                # ---- xt load + work = xt * Gg (elementwise, Pool engine) ----
                xt = xtp.tile([CIN, N], BF16, tag="xt")
                nc.sync.dma_start(xt, xtall_d[:, bsl])
                work = workp.tile([CIN, N], BF16, tag="work")
                nc.gpsimd.tensor_mul(work, xt, gg_s)

                # ---- f1b[j,i] = sum_c work[c,i] + kb'[i], bcast over 128 ----
                f1b = f1bp.tile([P, N], BF16, tag="f1b")
                for ch in range(8):
                    sl = slice(ch * 512, (ch + 1) * 512)
                    ps = psf1b.tile([P, 512], F32, tag="ps")
                    nc.tensor.matmul(ps, lhsT=ones128, rhs=work[:, sl],
                                     start=True, stop=False)
                    nc.tensor.matmul(ps, lhsT=ones1, rhs=kb_s[:, sl],
                                     start=False, stop=True)
                    nc.scalar.copy(f1b[:, sl], ps)

                # ---- hn (natural local h, biased) + f2 ----
                hn = hnp.tile([P, NJT * COUT], F32, tag="hn")
                f2cb = f2p.tile([P, NJT], F32, tag="f2")
                psn = psnp.tile([P, NJT * COUT], F32, tag="psn",
                                padded_shape=[P, 512])
                for jt in range(NJT):
                    osl = slice(jt * COUT, (jt + 1) * COUT)
                    xsl = slice(b * NJ + jt * P, b * NJ + (jt + 1) * P)
                    nc.tensor.matmul(psn[:, osl], lhsT=xthall_s[:, xsl],
                                     rhs=wpt_s, start=(jt == 0),
                                     stop=(jt == NJT - 1))
                    nc.vector.tensor_add(hn[:, osl], psn[:, osl], bpb_s)
                    pv = smalls.tile([P, COUT], F32, tag="pv")
                    nc.vector.tensor_mul(pv, hn[:, osl], vwn_s[:, osl])
                    f2r = smalls.tile([P, 1], F32, tag="f2r")
                    nc.vector.tensor_reduce(f2r, pv, axis=mybir.AxisListType.X,
                                            op=ALU.add)
                    nc.vector.tensor_add(f2cb[:, jt:jt + 1], f2r,
                                         vbcol_s[:, jt:jt + 1])

                # ---- elementwise chain + per-tile normalizers ----
                qt = qp.tile([P, NJT * COUT], BF16, tag="q")
                psg = psgp.tile([COUT, 1], F32, tag="psg",
                                padded_shape=[P, 512])
                tts = []
                for jt in range(NJT):
                    osl = slice(jt * COUT, (jt + 1) * COUT)
                    lt = lp.tile([P, N], BF16, tag="lt")
                    nc.vector.scalar_tensor_tensor(
                        lt, f1b, f2cb[:, jt:jt + 1], adjt_t[jt],
                        op0=ALU.add, op1=ALU.mult)
                    tt = tp.tile([P, N], BF16, tag="tt")
                    ts = smalls.tile([P, 1], F32, tag="ts")
                    nc.scalar.activation(tt, lt, AF.Tanh, bias=delta_col,
                                         scale=GAMMA, accum_out=ts)
                    dn = smalls.tile([P, 1], F32, tag="dn")
                    nc.vector.tensor_scalar_add(dn, ts, float(C0))
                    rc = smalls.tile([P, 1], F32, tag="rc")
                    nc.vector.reciprocal(rc, dn)
                    nc.vector.tensor_scalar_mul(qt[:, osl], hn[:, osl], rc)
                    nc.tensor.matmul(psg, lhsT=qt[:, osl], rhs=onescol,
                                     start=(jt == 0), stop=(jt == NJT - 1))
                    tts.append(tt)
                gv = smalls.tile([COUT, 1], F32, tag="gv")
                nc.vector.tensor_scalar_mul(gv, psg, float(AB))

                # ---- out^T[c,i] = sum_j q[j,c] t[j,i]  (+rank-1 on drain) ----
                out_sb = osb.tile([COUT, N], BF16, tag="osb")
                for ch in range(4):
                    pso = psop.tile([COUT, 1024], F32, tag="pso",
                                    padded_shape=[P, 1024])
                    for jt in range(NJT):
                        osl = slice(jt * COUT, (jt + 1) * COUT)
                        for hh in range(2):
                            isl = slice(ch * 1024 + hh * 512,
                                        ch * 1024 + (hh + 1) * 512)
                            nc.tensor.matmul(
                                pso[:, hh * 512:(hh + 1) * 512],
                                lhsT=qt[:, osl], rhs=tts[jt][:, isl],
                                start=(jt == 0), stop=(jt == NJT - 1))
                    nc.vector.tensor_scalar_add(
                        out_sb[:, ch * 1024:(ch + 1) * 1024], pso, gv)
                nc.sync.dma_start(outp_d[:, bsl], out_sb)

    nc.finalize()
    return nc


def _prep_in_maps(node_rep, adj_matrix, node_type, proj_W, proj_b,
                  k_W, k_b, v_W, v_b):
    """Host-side shard prep: layout, dtype casts, type-gathers, and
    N-independent parameter folding (G = k_W @ proj_W)."""
    f32 = np.float32
    x = np.asarray(node_rep, dtype=f32)
    adj = np.asarray(adj_matrix, dtype=f32)
    nt = np.asarray(node_type).astype(np.int64) % 5
    proj_W = np.asarray(proj_W, dtype=f32)
    proj_b = np.asarray(proj_b, dtype=f32)
    k_W = np.asarray(k_W, dtype=f32)
    k_b = np.asarray(k_b, dtype=f32)
    v_W = np.asarray(v_W, dtype=f32)
    v_b = np.asarray(v_b, dtype=f32)

    G5 = k_W @ proj_W                       # [5, CIN] folded f1 weights
    kbp = k_b + k_W @ proj_b                # [5] folded f1 bias
    gg = np.ascontiguousarray(G5[nt].T).astype(NPBF)          # [CIN, N]
    kbrow = kbp[nt][None, :].astype(NPBF)                     # [1, N]
    xtall = np.ascontiguousarray(
        x.transpose(2, 0, 1).reshape(CIN, B * N)).astype(NPBF)
    wpt = np.ascontiguousarray(proj_W.T).astype(NPBF)         # [CIN, COUT]
    bpb = np.ascontiguousarray(
        np.broadcast_to(proj_b[None, :], (P, COUT))).astype(f32)
    adjT = adj.T                            # adjT[j, i] = adj[i, j]
    VW = v_W[nt]                            # [N, COUT]
    vb = v_b[nt]                            # [N]

    in_maps = []
    for core in range(8):
        jsl = slice(core * NJ, (core + 1) * NJ)
        xth = np.ascontiguousarray(
            x[:, jsl, :].transpose(2, 0, 1).reshape(CIN, B * NJ)).astype(NPBF)
        vwn = np.ascontiguousarray(
            VW[jsl].reshape(NJT, P, COUT).transpose(1, 0, 2)
            .reshape(P, NJT * COUT)).astype(NPBF)
        vbcol = np.ascontiguousarray(vb[jsl].reshape(NJT, P).T).astype(f32)
        in_maps.append({
            "adjt": np.ascontiguousarray(adjT[jsl, :]).astype(NPBF),
            "xtall": xtall,
            "xthall": xth,
            "gg": gg,
            "kbrow": kbrow,
            "wpt": wpt,
            "bpb": bpb,
            "vwn": vwn,
            "vbcol": vbcol,
        })
    return in_maps


def kernel(node_rep, adj_matrix, node_type, proj_W, proj_b, k_W, k_b,
           v_W, v_b):
    global LAST_EXEC_NS, LAST_RESULTS
    in_maps = _prep_in_maps(node_rep, adj_matrix, node_type, proj_W,
                            proj_b, k_W, k_b, v_W, v_b)
    nc = build_nc()
    trace = os.environ.get("KERNEL_TRACE", "0") == "1"
    res = run_bass_kernel_spmd(nc, in_maps, core_ids=list(range(8)),
                               trace=trace)
    LAST_EXEC_NS = res.exec_time_ns
    LAST_RESULTS = res

    acc = None
    for core in range(8):
        part = np.asarray(res.results[core]["outp"]).astype(np.float32)
        acc = part if acc is None else acc + part
    # acc: [COUT, B*N] -> [B, N, COUT]
    return np.ascontiguousarray(
        acc.reshape(COUT, B, N).transpose(1, 2, 0))


# revision 6
# speedup vs baseline: 1.6982x; 1.2806x over previous
"""HGAT message-passing kernel for Trainium2 (8 NeuronCores, SPMD).

Reference computation (B=4, N=4096, C_IN=128, C_OUT=64):
    h   = node_rep @ proj_W.T + proj_b                    # [B,N,64]
    f1  = rowsum(h * k_W[node_type]) + k_b[node_type]     # [B,N]
    f2  = rowsum(h * v_W[node_type]) + v_b[node_type]     # [B,N]
    L   = adj[i,j] * (f1[i] + f2[j])
    P   = softmax(sigmoid(L) - 0.5, axis=i)   # normalized over i per col j
    out = P @ h                               # contract over j

Device algebra:
  * softmax over i / contract over j => out = E @ (h / colsum(E)) with
    E = exp(sigmoid(L)); -0.5 and max-subtraction cancel in the ratio.
  * E is ONE activation-table pass:  E ~ ALPHA + BETA*tanh(GAMMA*L + DELTA)
    (max rel err 5.5e-4).  With t = tanh(GAMMA*L+DELTA):
        colsum_j = ALPHA*N + BETA * sum_i t[i,j]        (tanh accum_out)
        out^T[c,i] = sum_j q[j,c] t[j,i] + (ALPHA/BETA) sum_j q[j,c]
    where q[j,c] = h[j,c] / (ALPHA*N/BETA + sum_i t[i,j]).  The rank-1
    term rides the PSUM->SBUF drain as a per-partition ACT bias.
  * f1 via folded weights G = k_W @ proj_W (host, 5x64x128 param-only):
    f1[i] = sum_c x[c,i]*Gg[c,i] (+kb'); the all-ones matmul reduces over
    c AND broadcasts across the 128 partitions in one PE pass.
  * elementwise L split as (f1b +s f2)*adj: tensor_scalar runs 4x-pumped,
    tensor_tensor 2x-pumped on the DVE (faster than one fused 1x stt).

Sharding: core c owns j-rows [c*512, (c+1)*512) of adj^T for ALL batches
and all i; the adjacency slice loads once (bf16) and is reused across the
4 batches; no collectives. Each core emits a [64, B*N] partial of out^T;
the host sums the 8 partials.  Zero biases (always zero in this problem's
setup_inputs) are specialized out at build time.
"""

import os
import sys

import numpy as np

sys.path.insert(0, "/opt/trn_rl_repo")

import ml_dtypes  # noqa: E402

import concourse.bass as bass  # noqa: E402
import concourse.tile as tile  # noqa: E402
from concourse import bacc  # noqa: E402
from concourse import mybir  # noqa: E402
from concourse.bass_utils import run_bass_kernel_spmd  # noqa: E402

B = 4
N = 4096
CIN = 128
COUT = 64
P = 128
NJ = N // 8            # 512 j-rows per core
NJT = NJ // P          # 4 j-tiles per core

# exp(sigmoid(x)) ~ ALPHA + BETA*tanh(GAMMA*x + DELTA), max rel err 5.5e-4
ALPHA = 1.8590168
BETA = 0.85887245
GAMMA = 0.5083613
DELTA = -0.2499283
C0 = ALPHA * N / BETA      # denom' = C0 + sum_i t   (q = h/denom')
AB = ALPHA / BETA          # rank-1 scale

F32 = mybir.dt.float32
BF16 = mybir.dt.bfloat16
AF = mybir.ActivationFunctionType
ALU = mybir.AluOpType
NPBF = ml_dtypes.bfloat16

LAST_EXEC_NS = None
LAST_RESULTS = None


def _view3(t, parts, d0, d1):
    """[parts, d0*d1] tile -> [parts, d0, d1] AP view."""
    return bass.AP(tensor=t.tensor, offset=t.offset,
                   ap=[[1, parts], [d1, d0], [1, d1]])


def build_nc(kb_zero=True, pb_zero=True, vb_zero=True):
    nc = bacc.Bacc()
    adjt_d = nc.dram_tensor("adjt", [NJ, N], BF16, kind="ExternalInput")
    xtall_d = nc.dram_tensor("xtall", [CIN, B * N], BF16, kind="ExternalInput")
    xthall_d = nc.dram_tensor("xthall", [CIN, B * NJ], BF16,
                              kind="ExternalInput")
    gg_d = nc.dram_tensor("gg", [CIN, N], BF16, kind="ExternalInput")
    wpt_d = nc.dram_tensor("wpt", [CIN, COUT], BF16, kind="ExternalInput")
    vwn_d = nc.dram_tensor("vwn", [P, NJT * COUT], BF16, kind="ExternalInput")
    outp_d = nc.dram_tensor("outp", [COUT, B * N], BF16, kind="ExternalOutput")
    if not kb_zero:
        kbrow_d = nc.dram_tensor("kbrow", [1, N], BF16, kind="ExternalInput")
    if not pb_zero:
        bpb_d = nc.dram_tensor("bpb", [P, COUT], F32, kind="ExternalInput")
    if not vb_zero:
        vbcol_d = nc.dram_tensor("vbcol", [P, NJT], F32, kind="ExternalInput")

    with tile.TileContext(nc) as tc:
        with (
            tc.tile_pool(name="singles", bufs=1) as singles,
            tc.tile_pool(name="xtp", bufs=2) as xtp,
            tc.tile_pool(name="workp", bufs=2) as workp,
            tc.tile_pool(name="f1bp", bufs=2) as f1bp,
            tc.tile_pool(name="fbp", bufs=2) as fbp,
            tc.tile_pool(name="lp", bufs=2) as lp,
            tc.tile_pool(name="tp", bufs=5) as tp,
            tc.tile_pool(name="qp", bufs=2) as qp,
            tc.tile_pool(name="f2p", bufs=2) as f2p,
            tc.tile_pool(name="osb", bufs=2) as osb,
            tc.tile_pool(name="smalls", bufs=4) as smalls,
            tc.tile_pool(name="psf1b", bufs=1, space="PSUM") as psf1b,
            tc.tile_pool(name="psn", bufs=1, space="PSUM") as psnp,
            tc.tile_pool(name="psg", bufs=1, space="PSUM") as psgp,
            tc.tile_pool(name="pso", bufs=2, space="PSUM") as psop,
        ):
            # ---- loads needed first (batch-0 critical path) ----
            xt0 = xtp.tile([CIN, N], BF16, tag="xt")
            nc.sync.dma_start(xt0, xtall_d[:, 0:N])
            gg_s = singles.tile([CIN, N], BF16, tag="gg")
            nc.sync.dma_start(gg_s, gg_d[:, :])
            wpt_s = singles.tile([CIN, COUT], BF16, tag="wpt")
            nc.sync.dma_start(wpt_s, wpt_d[:, :])
            xthall_s = singles.tile([CIN, B * NJ], BF16, tag="xth")
            nc.sync.dma_start(xthall_s, xthall_d[:, :])
            vwn_s = singles.tile([P, NJT * COUT], BF16, tag="vwn")
            nc.sync.dma_start(vwn_s, vwn_d[:, :])
            adjt_t = []
            for k in range(NJT):
                a = singles.tile([P, N], BF16, tag=f"adj{k}")
                nc.sync.dma_start(a, adjt_d[k * P:(k + 1) * P, :])
                adjt_t.append(a)
            if not kb_zero:
                kb_s = singles.tile([1, N], BF16, tag="kb")
                nc.sync.dma_start(kb_s, kbrow_d[:, :])
            if not pb_zero:
                bpb_s = singles.tile([P, COUT], F32, tag="bpb")
                nc.sync.dma_start(bpb_s, bpb_d[:, :])
            if not vb_zero:
                vbcol_s = singles.tile([P, NJT], F32, tag="vbc")
                nc.sync.dma_start(vbcol_s, vbcol_d[:, :])

            ones128 = singles.tile([P, P], BF16, tag="ones128")
            nc.vector.memset(ones128, 1.0)
            if not kb_zero:
                ones1 = singles.tile([1, P], BF16, tag="ones1")
                nc.vector.memset(ones1, 1.0)
            onescol = singles.tile([P, 1], BF16, tag="onescol")
            nc.vector.memset(onescol, 1.0)
            delta_col = singles.tile([P, 1], F32, tag="delta")
            nc.vector.memset(delta_col, DELTA)

            for b in range(B):
                bsl = slice(b * N, (b + 1) * N)
                # ---- xt load + work = xt * Gg (DVE 2x) ----
                if b == 0:
                    xt = xt0
                else:
                    xt = xtp.tile([CIN, N], BF16, tag="xt")
                    nc.sync.dma_start(xt, xtall_d[:, bsl])
                work = workp.tile([CIN, N], BF16, tag="work")
                nc.vector.tensor_mul(work, xt, gg_s)

                # ---- f1b[j,i] = sum_c work[c,i] (+kb'), bcast over 128 ----
                f1b = f1bp.tile([P, N], BF16, tag="f1b")
                for ch in range(4):
                    ps = psf1b.tile([P, 1024], F32, tag="ps",
                                    padded_shape=[P, 1024])
                    for hh in range(2):
                        sl = slice(ch * 1024 + hh * 512,
                                   ch * 1024 + (hh + 1) * 512)
                        psl = ps[:, hh * 512:(hh + 1) * 512]
                        nc.tensor.matmul(psl, lhsT=ones128, rhs=work[:, sl],
                                         start=True, stop=kb_zero)
                        if not kb_zero:
                            nc.tensor.matmul(psl, lhsT=ones1, rhs=kb_s[:, sl],
                                             start=False, stop=True)
                    dst = f1b[:, ch * 1024:(ch + 1) * 1024]
                    if ch < 3:
                        nc.scalar.copy(dst, ps)
                    else:
                        nc.vector.tensor_copy(dst, ps)

                # ---- hn (natural local h) + f2 ----
                psn = psnp.tile([P, NJT * COUT], F32, tag="psn",
                                padded_shape=[P, 512])
                for jt in range(NJT):
                    osl = slice(jt * COUT, (jt + 1) * COUT)
                    xsl = slice(b * NJ + jt * P, b * NJ + (jt + 1) * P)
                    nc.tensor.matmul(psn[:, osl], lhsT=xthall_s[:, xsl],
                                     rhs=wpt_s, start=(jt == 0),
                                     stop=(jt == NJT - 1))
                if pb_zero:
                    hn = psn
                else:
                    hn = smalls.tile([P, NJT * COUT], F32, tag="hn")
                    nc.vector.tensor_add(hn, psn, bpb_s)
                pv = smalls.tile([P, NJT * COUT], F32, tag="pv")
                nc.vector.tensor_mul(pv, hn, vwn_s)
                f2cb = f2p.tile([P, NJT], F32, tag="f2")
                for jt in range(NJT):
                    nc.vector.tensor_reduce(
                        f2cb[:, jt:jt + 1], pv[:, jt * COUT:(jt + 1) * COUT],
                        axis=mybir.AxisListType.X, op=ALU.add)
                if not vb_zero:
                    nc.vector.tensor_add(f2cb, f2cb, vbcol_s)

                # ---- elementwise chain + per-tile normalizers ----
                qt = qp.tile([P, NJT * COUT], BF16, tag="q")
                psg = psgp.tile([COUT, 1], F32, tag="psg",
                                padded_shape=[P, 512])
                tts = []
                for jt in range(NJT):
                    osl = slice(jt * COUT, (jt + 1) * COUT)
                    fb = fbp.tile([P, N], BF16, tag="fb")
                    nc.vector.tensor_scalar_add(fb, f1b, f2cb[:, jt:jt + 1])
                    lt = lp.tile([P, N], BF16, tag="lt")
                    nc.vector.tensor_mul(lt, fb, adjt_t[jt])
                    tt = tp.tile([P, N], BF16, tag="tt")
                    ts = smalls.tile([P, 1], F32, tag="ts")
                    nc.scalar.activation(tt, lt, AF.Tanh, bias=delta_col,
                                         scale=GAMMA, accum_out=ts)
                    dn = smalls.tile([P, 1], F32, tag="dn")
                    nc.vector.tensor_scalar_add(dn, ts, float(C0))
                    rc = smalls.tile([P, 1], F32, tag="rc")
                    nc.vector.reciprocal(rc, dn)
                    nc.vector.tensor_scalar_mul(qt[:, osl], hn[:, osl], rc)
                    nc.tensor.matmul(psg, lhsT=qt[:, osl], rhs=onescol,
                                     start=(jt == 0), stop=(jt == NJT - 1))
                    tts.append(tt)
                gv = smalls.tile([COUT, 1], F32, tag="gv")
                nc.vector.tensor_scalar_mul(gv, psg, float(AB))

                # ---- out^T[c,i] = sum_j q[j,c] t[j,i] (+rank-1 on drain) ----
                out_sb = osb.tile([COUT, N], BF16, tag="osb")
                for ch in range(4):
                    pso = psop.tile([COUT, 1024], F32, tag="pso",
                                    padded_shape=[P, 1024])
                    for jt in range(NJT):
                        osl = slice(jt * COUT, (jt + 1) * COUT)
                        for hh in range(2):
                            isl = slice(ch * 1024 + hh * 512,
                                        ch * 1024 + (hh + 1) * 512)
                            nc.tensor.matmul(
                                pso[:, hh * 512:(hh + 1) * 512],
                                lhsT=qt[:, osl], rhs=tts[jt][:, isl],
                                start=(jt == 0), stop=(jt == NJT - 1))
                    dst = out_sb[:, ch * 1024:(ch + 1) * 1024]
                    if ch < 3:
                        nc.scalar.activation(dst, pso, AF.Identity, bias=gv,
                                             scale=1.0)
                    else:
                        nc.vector.tensor_scalar_add(dst, pso, gv)
                nc.sync.dma_start(outp_d[:, bsl], out_sb)

    nc.finalize()
    return nc


def _prep_in_maps(node_rep, adj_matrix, node_type, proj_W, proj_b,
                  k_W, k_b, v_W, v_b):
    """Host-side shard prep: layout, dtype casts, type-gathers, and
    N-independent parameter folding (G = k_W @ proj_W)."""
    f32 = np.float32
    x = np.asarray(node_rep, dtype=f32)
    adj = np.asarray(adj_matrix, dtype=f32)
    nt = np.asarray(node_type).astype(np.int64) % 5
    proj_W = np.asarray(proj_W, dtype=f32)
    proj_b = np.asarray(proj_b, dtype=f32)
    k_W = np.asarray(k_W, dtype=f32)
    k_b = np.asarray(k_b, dtype=f32)
    v_W = np.asarray(v_W, dtype=f32)
    v_b = np.asarray(v_b, dtype=f32)

    G5 = k_W @ proj_W                       # [5, CIN] folded f1 weights
    kbp = k_b + k_W @ proj_b                # [5]  folded f1 bias
    kb_zero = bool(np.all(kbp == 0.0))
    pb_zero = bool(np.all(proj_b == 0.0))
    vb_zero = bool(np.all(v_b == 0.0))

    gg = np.ascontiguousarray(G5[nt].T).astype(NPBF)          # [CIN, N]
    xtall = np.ascontiguousarray(
        x.transpose(2, 0, 1).reshape(CIN, B * N)).astype(NPBF)
    wpt = np.ascontiguousarray(proj_W.T).astype(NPBF)         # [CIN, COUT]
    adjT = adj.T                            # adjT[j, i] = adj[i, j]
    VW = v_W[nt]                            # [N, COUT]
    vb = v_b[nt]                            # [N]

    in_maps = []
    for core in range(8):
        jsl = slice(core * NJ, (core + 1) * NJ)
        xth = np.ascontiguousarray(
            x[:, jsl, :].transpose(2, 0, 1).reshape(CIN, B * NJ)).astype(NPBF)
        vwn = np.ascontiguousarray(
            VW[jsl].reshape(NJT, P, COUT).transpose(1, 0, 2)
            .reshape(P, NJT * COUT)).astype(NPBF)
        m = {
            "adjt": np.ascontiguousarray(adjT[jsl, :]).astype(NPBF),
            "xtall": xtall,
            "xthall": xth,
            "gg": gg,
            "wpt": wpt,
            "vwn": vwn,
        }
        if not kb_zero:
            m["kbrow"] = kbp[nt][None, :].astype(NPBF)
        if not pb_zero:
            m["bpb"] = np.ascontiguousarray(
                np.broadcast_to(proj_b[None, :], (P, COUT))).astype(f32)
        if not vb_zero:
            m["vbcol"] = np.ascontiguousarray(
                vb[jsl].reshape(NJT, P).T).astype(f32)
        in_maps.append(m)
    return in_maps, kb_zero, pb_zero, vb_zero


def kernel(node_rep, adj_matrix, node_type, proj_W, proj_b, k_W, k_b,
           v_W, v_b):
    global LAST_EXEC_NS, LAST_RESULTS
    in_maps, kb_zero, pb_zero, vb_zero = _prep_in_maps(
        node_rep, adj_matrix, node_type, proj_W, proj_b, k_W, k_b, v_W, v_b)
    nc = build_nc(kb_zero=kb_zero, pb_zero=pb_zero, vb_zero=vb_zero)
    trace = os.environ.get("KERNEL_TRACE", "0") == "1"
    res = run_bass_kernel_spmd(nc, in_maps, core_ids=list(range(8)),
                               trace=trace)
    LAST_EXEC_NS = res.exec_time_ns
    LAST_RESULTS = res

    acc = None
    for core in range(8):
        part = np.asarray(res.results[core]["outp"]).astype(np.float32)
        acc = part if acc is None else acc + part
    # acc: [COUT, B*N] -> [B, N, COUT]
    return np.ascontiguousarray(
        acc.reshape(COUT, B, N).transpose(1, 2, 0))
